# revision 1
# baseline (speedup 1.0000x reference)
"""Multi-head self-attention (N=2, S=2048, E=1024, 16 heads) on 8 trn2 cores.

Sharding: data parallel over batch (2) x tensor parallel over heads (4 groups
of 4 heads). Each core computes in_proj for its local heads, attention with
full SxS scores for its local heads, and a partial out_proj (contraction over
its local 256 features). Host sums the 4 partials per batch and adds b_o.

Device kernel (per core), all matmuls in float32r (TF32-like, full PE rate):
  phase 1: qT/kT = W_{q,k} @ x^T  (features on partitions), V natural layout
           with a ones-column appended per head (softmax denominators).
  phase 2: scores transposed sT[k, q] = K Q^T per 128-k tile; exp on ACT;
           attnV out^T[d, q] accumulated over k, the ones row yielding
           sum_k exp; divide via reciprocal + gpsimd partition_broadcast +
           DVE multiply; out_proj fused per query block.
"""
import numpy as np

import concourse.bacc as bacc
import concourse.mybir as mybir
from concourse.tile import TileContext
from concourse.bass import ts

F32 = mybir.dt.float32
F32R = mybir.dt.float32r
EXP = mybir.ActivationFunctionType.Exp

D_MODEL = 1024
NHEAD = 16
DH = 64
N_BATCH = 2
SEQ = 2048
N_CORES = 8
GROUPS = 4            # head groups (cores per batch)
HL = NHEAD // GROUPS  # local heads per core = 4
FL = HL * DH          # local feature width = 256


def build_mha(nc, S=SEQ, E=D_MODEL, EOUT=D_MODEL, HLOC=HL, scale=0.125):
    """Emit the per-core kernel IR. Returns nothing; declares DRAM I/O."""
    FLOC = HLOC * DH          # local q/k/v feature count
    EC = E // 128             # contraction chunks for in_proj
    FT = FLOC // 128          # feature tiles for qT/kT (heads per tile = 2)
    TT = S // 128             # token tiles
    QB = S // 512             # 512-wide query blocks
    KT = S // 128             # 128-wide key tiles
    OC = FLOC // 128          # out_proj contraction chunks
    EB = (EOUT + 511) // 512  # out_proj output blocks
    TPQ = TT // QB            # token tiles per query block (4)

    xT = nc.dram_tensor("xT", [E, S], F32R, kind="ExternalInput")
    wT = nc.dram_tensor("wT", [E, 3 * FLOC], F32R, kind="ExternalInput")
    qkb = nc.dram_tensor("qkb", [128, 2 * FT], F32, kind="ExternalInput")
    vbr = nc.dram_tensor("vbr", [128, FLOC], F32, kind="ExternalInput")
    woT = nc.dram_tensor("woT", [FLOC, EOUT], F32R, kind="ExternalInput")
    vones = nc.dram_tensor("vones", [128, TT * HLOC], F32R, kind="ExternalInput")
    out = nc.dram_tensor("out", [S, EOUT], F32, kind="ExternalOutput")

    with TileContext(nc) as tc:
        with tc.tile_pool(name="persist", bufs=1) as pp:
            qkb_sb = pp.tile([128, 2 * FT], F32)
            nc.sync.dma_start(qkb_sb[:], qkb[:])
            vbr_sb = pp.tile([128, FLOC], F32)
            nc.sync.dma_start(vbr_sb[:], vbr[:])
            qT = pp.tile([128, FT, S], F32R)
            kT = pp.tile([128, FT, S], F32R)
            v = pp.tile([128, TT, HLOC, 65], F32R)
            outT = pp.tile([128, OC, S], F32R)
            woT_sb = pp.tile([128, OC, EOUT], F32R)

            with tc.tile_pool(name="ph2ps", bufs=2, space="PSUM") as ps2:
                # ---- phase 1: in_proj ----
                with tc.tile_pool(name="ph1", bufs=1) as p1, \
                     tc.tile_pool(name="ph1ps", bufs=2, space="PSUM") as ps1:
                    xT_sb = p1.tile([128, EC, S], F32R)
                    wT_sb = p1.tile([128, EC, 3 * FLOC], F32R)
                    for c in range(EC):
                        nc.sync.dma_start(wT_sb[:, c, 0:2 * FLOC],
                                          wT[ts(c, 128), 0:2 * FLOC])
                        nc.sync.dma_start(xT_sb[:, c, 0:512], xT[ts(c, 128), 0:512])
                    for tb in range(1, S // 512):
                        for c in range(EC):
                            nc.sync.dma_start(xT_sb[:, c, ts(tb, 512)],
                                              xT[ts(c, 128), ts(tb, 512)])
                    for c in range(EC):
                        nc.sync.dma_start(wT_sb[:, c, 2 * FLOC:],
                                          wT[ts(c, 128), 2 * FLOC:])
                    for c in range(OC):
                        nc.sync.dma_start(woT_sb[:, c, :], woT[ts(c, 128), :])
                    nc.sync.dma_start(
                        v[:, :, :, 64:65],
                        vones.rearrange("p (t h one) -> p t h one", h=HLOC, one=1))

                    # q/k first so kT/qT complete early and attention can begin
                    # while the V projection still runs
                    for tb in range(S // 512):
                        for ft in range(FT):
                            for gi, (dst, off) in enumerate(((qT, 0), (kT, FLOC))):
                                pq = ps1.tile([128, 512], F32, tag="pq")
                                lo = off + ft * 128
                                for c in range(EC):
                                    nc.tensor.matmul(
                                        pq[:], wT_sb[:, c, lo:lo + 128],
                                        xT_sb[:, c, ts(tb, 512)],
                                        start=(c == 0), stop=(c == EC - 1))
                                nc.vector.tensor_scalar_add(
                                    dst[:, ft, ts(tb, 512)], pq[:],
                                    qkb_sb[:, gi * FT + ft:gi * FT + ft + 1])
                    # V natural layout: [tok, vfeat] per 128-token tile
                    for t in range(TT):
                        pv = ps1.tile([128, FLOC], F32, tag="pv")
                        for c in range(EC):
                            nc.tensor.matmul(
                                pv[:], xT_sb[:, c, ts(t, 128)],
                                wT_sb[:, c, 2 * FLOC:3 * FLOC],
                                start=(c == 0), stop=(c == EC - 1))
                        nc.vector.tensor_add(
                            v[:, t, :, 0:64],
                            pv.rearrange("p (h d) -> p h d", h=HLOC),
                            vbr_sb.rearrange("p (h d) -> p h d", h=HLOC))

                # ---- phase 2+3: attention with fused out_proj per query block ----
                with tc.tile_pool(name="ph2", bufs=16) as p2, \
                     tc.tile_pool(name="ph2oc", bufs=2) as p2oc, \
                     tc.tile_pool(name="ph3", bufs=2) as p3, \
                     tc.tile_pool(name="ph2po", bufs=1, space="PSUM") as ps2o, \
                     tc.tile_pool(name="ph3ps", bufs=1, space="PSUM") as ps3:
                    onum = 2
                    for qb in range(QB):
                        for hp in range(HLOC // 2):
                            o0 = ps2o.tile([65, 512], F32, tag=f"o{onum % 3}")
                            o1 = ps2o.tile([65, 512], F32, tag=f"o{(onum + 1) % 3}")
                            onum += 2
                            oo = [o0, o1]
                            for kt in range(KT):
                                sps = ps2.tile([128, 2, 512], F32, tag="s")
                                ex = p2.tile([128, 2, 512], F32R, tag="exp")
                                for hh in range(2):
                                    p0 = 64 * hh
                                    nc.tensor.matmul(
                                        sps[:, hh, :],
                                        kT[p0:p0 + 64, hp, ts(kt, 128)],
                                        qT[p0:p0 + 64, hp, ts(qb, 512)],
                                        start=True, stop=True)
                                nc.scalar.activation(ex[:], sps[:], EXP, scale=scale)
                                for hh in range(2):
                                    nc.tensor.matmul(
                                        oo[hh][:],
                                        v[:, kt, 2 * hp + hh, :],
                                        ex[:, hh, :],
                                        start=(kt == 0),
                                        stop=(kt == KT - 1))
                            for hh in range(2):
                                # copy out of PSUM promptly so the o slot frees for
                                # the next head pair; divide from the SBUF copy
                                rec = p2oc.tile([1, 512], F32, tag="rec")
                                nc.vector.reciprocal(rec[:], oo[hh][64:65, :])
                                oc = p2oc.tile([65, 512], F32, tag="oc")
                                nc.vector.tensor_copy(oc[0:64, :], oo[hh][0:64, :])
                                rep = p2oc.tile([64, 512], F32, tag="rep")
                                nc.gpsimd.partition_broadcast(rep[:], rec[:])
                                # outT chunk hp holds feats of heads (2hp, 2hp+1)
                                nc.vector.tensor_mul(
                                    outT[64 * hh:64 * hh + 64, hp, ts(qb, 512)],
                                    oc[0:64, :], rep[:])
                        # out_proj for this query block's token tiles
                        for t in range(TPQ * qb, TPQ * qb + TPQ):
                            fo = p3.tile([128, EOUT], F32, tag="fo")
                            for eb in range(EB):
                                w = min(512, EOUT - eb * 512)
                                po = ps3.tile([128, 512], F32, tag="po")
                                for c in range(OC):
                                    nc.tensor.matmul(
                                        po[:, :w], outT[:, c, ts(t, 128)],
                                        woT_sb[:, c, eb * 512:eb * 512 + w],
                                        start=(c == 0), stop=(c == OC - 1))
                                nc.vector.tensor_copy(fo[:, eb * 512:eb * 512 + w],
                                                      po[:, :w])
                            nc.sync.dma_start(out[ts(t, 128), :], fo[:])


_CACHED = {}


def _get_module():
    if "nc" not in _CACHED:
        nc = bacc.Bacc("TRN2")
        build_mha(nc)
        nc.finalize()
        _CACHED["nc"] = nc
    return _CACHED["nc"]


def make_in_maps(query, w_in, b_in, w_o):
    """Host-side sharding: per-core input dicts (layout transforms included)."""
    E, HLoc, FLoc = D_MODEL, HL, FL
    woT_full = np.ascontiguousarray(w_o.T, dtype=np.float32)  # (e_in, e_out)
    vones_arr = np.ones((128, (SEQ // 128) * HLoc), np.float32)
    in_maps = []
    for core in range(N_CORES):
        b, g = divmod(core, GROUPS)
        rows = np.r_[g * FLoc:(g + 1) * FLoc,
                     E + g * FLoc:E + (g + 1) * FLoc,
                     2 * E + g * FLoc:2 * E + (g + 1) * FLoc]
        bl = b_in[rows].astype(np.float32)
        ft_n = FLoc // 128
        qkb_c = np.empty((128, 2 * ft_n), np.float32)
        for ft in range(ft_n):
            qkb_c[:, ft] = bl[ft * 128:(ft + 1) * 128]
            qkb_c[:, ft_n + ft] = bl[FLoc + ft * 128:FLoc + (ft + 1) * 128]
        vbr_c = np.ascontiguousarray(
            np.broadcast_to(bl[2 * FLoc:], (128, FLoc)))
        in_maps.append({
            "vones": vones_arr,
            "xT": np.ascontiguousarray(query[b].T, dtype=np.float32),
            "wT": np.ascontiguousarray(w_in[rows].T, dtype=np.float32),
            "qkb": qkb_c, "vbr": vbr_c,
            "woT": np.ascontiguousarray(woT_full[g * FLoc:(g + 1) * FLoc]),
        })
    return in_maps


def kernel(query, key, value, w_in, b_in, w_o, b_o, _trace=False):
    from concourse.bass_utils import run_bass_kernel_spmd
    query = np.asarray(query, dtype=np.float32)
    nc = _get_module()
    in_maps = make_in_maps(query, np.asarray(w_in), np.asarray(b_in),
                           np.asarray(w_o))
    res = run_bass_kernel_spmd(nc, in_maps, core_ids=list(range(N_CORES)),
                               trace=_trace)
    out = np.empty((N_BATCH, SEQ, D_MODEL), np.float32)
    for b in range(N_BATCH):
        acc = res.results[b * GROUPS]["out"].astype(np.float32)
        for g in range(1, GROUPS):
            acc = acc + res.results[b * GROUPS + g]["out"]
        out[b] = acc + np.asarray(b_o, dtype=np.float32)[None, :]
    if _trace:
        kernel.last_exec_time_ns = res.exec_time_ns
    return out



# revision 3
# speedup vs baseline: 1.1045x; 1.1045x over previous
"""MHA (N=2, S=2048, E=1024, 16 heads) on 8 trn2 cores — fp8-hilo/DR design.

Per core: 1 batch x 4 heads.
  - in_proj: fp8 hi/lo 3-term (xh*wh + xh*wl + xl*wh) DoubleRow matmuls.
    q bias added via a 1-partition ones matmul (k bias cancels in softmax,
    v bias added at the cast).
  - q/k cast to fp8 hi/lo (2 engine ops per tile), fold-DMAd into
    K-stack [kh;kl] and Q-dup ([qh;qh],[ql;ql]) layouts.
  - scores: one DoubleRow matmul per (head, kt): full (kh+kl)(qh+ql).
  - exp: ACT exact / DVE/Pool Schraudolph-bf16 (u16 bit trick), per-tile
    engine assignment pattern.
  - attnV: bf16, out[q, 65] with a ones column giving softmax denominators.
  - divide: DVE reciprocal + gpsimd scalar mults -> attnout bf16.
  - out_proj: bf16 after DMA-transpose of attnout; fp32 partial out.
Host sums the 4 head-group partials per batch and adds b_o.
"""
import numpy as np

import concourse.bacc as bacc
import concourse.mybir as mybir
from concourse.tile import TileContext
from concourse.bass import ts

F32 = mybir.dt.float32
F8 = mybir.dt.float8e4
U16 = mybir.dt.uint16
BF16 = mybir.dt.bfloat16
EXP = mybir.ActivationFunctionType.Exp
COPY = mybir.ActivationFunctionType.Copy
DR = mybir.MatmulPerfMode.DoubleRow
MULT = mybir.AluOpType.mult
ADD = mybir.AluOpType.add
SUB = mybir.AluOpType.subtract

D_MODEL = 1024
NHEAD = 16
DH = 64
N_BATCH = 2
SEQ = 2048
N_CORES = 8
GROUPS = 4
HL = NHEAD // GROUPS   # 4 local heads
FL = HL * DH           # 256 local features

# ---- scales (powers of two) ----
SX = 8.0               # x scale before fp8 hilo
SWQ = 4096.0           # (wq*0.125) scale
SWK = 512.0            # wk scale
SWV = 512.0            # wv scale
CQ = 2.0 ** -7         # q psum -> fp8 cast scale
CK = 2.0 ** -9         # k psum -> fp8 cast scale
AQ = SX * SWQ * CQ     # q8 = q_true(incl 0.125) * AQ = 256
AK = SX * SWK * CK     # k8 = k_true * AK = 8
SE = 1.0 / (AQ * AK)   # exp arg scale on raw scores
VDESC = 1.0 / (SX * SWV)  # v descale folded into div-mult

SCH_A = (128.0 / np.log(2.0)) * SE
SCH_B = 127.0 * 128.0 + 0.5

# exp engine per tile: a=ACT exact, d=DVE, p=Pool schraudolph (balanced)
def _mk_exp_pat(na=140, nd=116, np_=0):
    tot = na + nd + np_
    acc = {"a": 0.0, "d": 0.0, "p": 0.0}
    w = {"a": na / tot, "d": nd / tot, "p": np_ / tot}
    pat = []
    for _ in range(tot):
        for k in acc:
            acc[k] += w[k]
        best = max(acc, key=lambda k: acc[k])
        acc[best] -= 1.0
        pat.append(best)
    return "".join(pat)

EXP_PAT = _mk_exp_pat()


def build_mha(nc, S=SEQ, E=D_MODEL, HLOC=HL):
    FLOC = HLOC * DH      # 256
    ECP = E // 256        # 4 contraction chunk pairs (DR fold)
    TT = S // 128         # 16 token tiles
    TB = S // 512         # 4 token blocks
    QB = S // 512         # 4 query blocks
    FT = FLOC // 128      # 2 feature tiles for q (and k)

    xh = nc.dram_tensor("xh", [128, ECP, 2, S], F8, kind="ExternalInput")
    xl = nc.dram_tensor("xl", [128, ECP, 2, S], F8, kind="ExternalInput")
    wh = nc.dram_tensor("wh", [128, ECP, 2, 3 * FLOC], F8, kind="ExternalInput")
    wl = nc.dram_tensor("wl", [128, ECP, 2, 3 * FLOC], F8, kind="ExternalInput")
    qb8 = nc.dram_tensor("qb8", [1, FLOC], F8, kind="ExternalInput")
    ones8 = nc.dram_tensor("ones8", [1, 512], F8, kind="ExternalInput")
    vb8 = nc.dram_tensor("vb8", [1, FLOC], F8, kind="ExternalInput")
    woT = nc.dram_tensor("woT", [128, FT, E], BF16, kind="ExternalInput")
    out = nc.dram_tensor("out", [S, E], F32, kind="ExternalOutput")

    with TileContext(nc) as tc:
        with tc.tile_pool(name="pp", bufs=1) as pp, \
             tc.tile_pool(name="stg", bufs=1) as stg, \
             tc.tile_pool(name="exb", bufs=1) as exb, \
             tc.tile_pool(name="osb", bufs=1) as osb, \
             tc.tile_pool(name="ps2", bufs=1, space="PSUM") as ps2:
            xh_sb = pp.tile([128, ECP, 2, S], F8)
            xl_sb = pp.tile([128, ECP, 2, S], F8)
            wh_sb = pp.tile([128, ECP, 2, 3 * FLOC], F8)
            wl_sb = pp.tile([128, ECP, 2, 3 * FLOC], F8)
            qb_sb = pp.tile([1, FLOC], F8)
            on_sb = pp.tile([1, 512], F8)
            vb_sb = pp.tile([1, FLOC], F8)
            woT_sb = pp.tile([128, FT, E], BF16)
            kst = pp.tile([128, HLOC, S], F8)         # [kh;kl] per head
            qdup = pp.tile([128, HLOC, 2, S], F8)     # [qh;qh],[ql;ql]
            vsb = pp.tile([128, TT, HLOC, 68], BF16)  # v + 1/VDESC col @64
            aoT = pp.tile([128, FT, TT, 128], BF16)   # attnout transposed
            rec = pp.tile([128, TT, HLOC], F32)       # VDESC/denominator
            stages = {}
            for ftile in range(2 * FT):
                for pl in ("hi", "lo"):
                    stages[(ftile, pl)] = stg.tile(
                        [128, TB, 512], F8, name=f"st{ftile}{pl}")

            nc.gpsimd.memset(vsb[:, :, :, 64:65], 1.0 / VDESC)
            nc.sync.dma_start(qb_sb[:], qb8[:])
            nc.sync.dma_start(on_sb[:], ones8[:])
            nc.sync.dma_start(vb_sb[:], vb8[:])
            for c in range(ECP):
                nc.sync.dma_start(wh_sb[:, c], wh[:, c])
                nc.sync.dma_start(xh_sb[:, c], xh[:, c])
                nc.sync.dma_start(wl_sb[:, c], wl[:, c])
                nc.sync.dma_start(xl_sb[:, c], xl[:, c])
            for ft in range(FT):
                nc.sync.dma_start(woT_sb[:, ft], woT[:, ft])

            TERMS = ((xh_sb, wh_sb), (xh_sb, wl_sb), (xl_sb, wh_sb))
            scc = [0]
            ei = [0]
            touches = {"av0": 0, "av1": 0, "den": 0}
            LAG = 6

            def next_sc(name):
                scc[0] += 1
                return ps2.tile([128, 512], F32, tag=f"sc{scc[0] % 5}",
                                name=name)

            def proj_row(ftile):
                """in_proj for one 128-feature row (q or k), chunk-major
                across the 4 token blocks, then casts + fold DMAs."""
                isq = ftile < FT
                fo = ftile * 128
                pqs = [next_sc(f"pq{tb}_{ftile}") for tb in range(TB)]
                for ti, (xa, wa) in enumerate(TERMS):
                    for c in range(ECP):
                        for tb in range(TB):
                            nc.tensor.matmul(
                                pqs[tb][:], wa[:, c, :, fo:fo + 128],
                                xa[:, c, :, ts(tb, 512)],
                                start=(ti == 0 and c == 0),
                                stop=(not isq and ti == 2 and c == ECP - 1),
                                perf_mode=DR)
                for tb in range(TB):
                    pq = pqs[tb]
                    if isq:
                        nc.tensor.matmul(pq[:], qb_sb[:, fo:fo + 128],
                                         on_sb[:], start=False, stop=True)
                    cs = CQ if isq else CK
                    hi = stages[(ftile, "hi")][:, tb, :]
                    lo = stages[(ftile, "lo")][:, tb, :]
                    nc.scalar.activation(hi, pq[:], COPY, scale=cs)
                    nc.vector.scalar_tensor_tensor(lo, pq[:], cs, hi,
                                                   MULT, SUB)
                for h2 in range(2):
                    h = (ftile % FT) * 2 + h2
                    s0, s1 = 64 * h2, 64 * h2 + 64
                    hi = stages[(ftile, "hi")]
                    lo = stages[(ftile, "lo")]
                    if isq:
                        for pl, srct in ((0, hi), (1, lo)):
                            for dup in range(2):
                                nc.sync.dma_start(
                                    qdup[64 * dup:64 * dup + 64, h, pl, :],
                                    srct[s0:s1])
                    else:
                        nc.sync.dma_start(kst[0:64, h, :], hi[s0:s1])
                        nc.sync.dma_start(kst[64:128, h, :], lo[s0:s1])

            def emit_v(t):
                pv = next_sc(f"pv{t}")
                for ti, (xa, wa) in enumerate(TERMS):
                    for c in range(ECP):
                        nc.tensor.matmul(
                            pv[:, 0:FLOC], xa[:, c, :, ts(t, 128)],
                            wa[:, c, :, 2 * FLOC:3 * FLOC],
                            start=(ti == 0 and c == 0), stop=False,
                            perf_mode=DR)
                nc.tensor.matmul(pv[:, 0:FLOC], on_sb[:, 0:128], vb_sb[:],
                                 start=False, stop=True)
                pvv = pv[:, 0:FLOC].rearrange("p (h d) -> p h d", h=HLOC)
                if t % 2 == 0:
                    nc.scalar.activation(vsb[:, t, :, 0:64], pvv, COPY)
                else:
                    nc.vector.tensor_copy(vsb[:, t, :, 0:64], pvv)

            def emit_po(qb, qt, i):
                qtg = 4 * qb + qt
                osf = osb.tile([128, 2, 512], F32, tag=f"os{qt % 2}",
                               name=f"os{qb}_{qt}")
                for eb in range(2):
                    po = next_sc(f"po{qb}_{qt}_{eb}")
                    for fc in range(FT):
                        nc.tensor.matmul(
                            po[:], aoT[:, fc, qtg, :],
                            woT_sb[:, fc, ts(eb, 512)],
                            start=(fc == 0), stop=(fc == FT - 1))
                    if eb == 0:
                        nc.scalar.activation(osf[:, eb, :], po[:], COPY)
                    else:
                        nc.vector.tensor_copy(osf[:, eb, :], po[:])
                nc.sync.dma_start(out[ts(qtg, 128), :],
                                  osf.rearrange("p a b -> p (a b)")[:])

            def run_tiles(qb, tiles, avp, den, extra=None):
                """software-pipelined scores+exp / attnV over tile list."""
                exs = {}
                for i in range(len(tiles) + LAG):
                    if extra:
                        extra(i)
                    if i < len(tiles):
                        kt, h = tiles[i]
                        sc = next_sc(f"sc{qb}_{kt}_{h}")
                        lhsT = kst[:, h, ts(kt, 128)].rearrange(
                            "p (one k) -> p one k",
                            one=1).broadcast_to([128, 2, 128])
                        nc.tensor.matmul(sc[:], lhsT,
                                         qdup[:, h, :, ts(qb, 512)],
                                         start=True, stop=True, perf_mode=DR)
                        ex = exb.tile([128, 512], BF16, tag=f"ex{i % 8}",
                                      name=f"ex{qb}_{kt}_{h}")
                        mode = EXP_PAT[ei[0] % len(EXP_PAT)]
                        ei[0] += 1
                        if mode == "a":
                            nc.scalar.activation(ex[:], sc[:], EXP, scale=SE)
                        else:
                            nc.vector.tensor_scalar(
                                ex.bitcast(U16)[:], sc[:], SCH_A, SCH_B,
                                MULT, ADD)
                        exs[i] = ex
                    if i >= LAG and i - LAG < len(tiles):
                        kt, h = tiles[i - LAG]
                        ex = exs.pop(i - LAG)
                        for qt in range(4):
                            bk = f"av{qt // 2}"
                            touches[bk] += 1
                            nc.tensor.matmul(
                                avp[qt // 2][:, 4 * (qt % 2) + h, :],
                                ex[:, ts(qt, 128)],
                                vsb[:, kt, h, 0:64],
                                start=(touches[bk] == 1),
                                stop=(touches[bk] == 2 * len(tiles) * 2))
                            touches["den"] += 1
                            nc.tensor.matmul(
                                den[:, 4 * qt + h:4 * qt + h + 1],
                                ex[:, ts(qt, 128)],
                                vsb[:, kt, h, 64:65],
                                start=(touches["den"] == 1),
                                stop=(touches["den"] == 4 * len(tiles) * 2))

            def qb_tail(qb, avp, den):
                nc.vector.reciprocal(rec[:, 4 * qb:4 * qb + 4, :]
                                     .rearrange("p a b -> p (a b)"), den[:])
                for qt in range(4):
                    qtg = 4 * qb + qt
                    ao = osb.tile([128, 256], BF16, tag=f"ao{qt % 2}",
                                  name=f"ao{qb}_{qt}")
                    for h in range(HLOC):
                        if h % 2:
                            nc.vector.tensor_scalar(
                                ao[:, ts(h, 64)],
                                avp[qt // 2][:, 4 * (qt % 2) + h, :],
                                rec[:, qtg, h:h + 1], None, MULT)
                        else:
                            nc.scalar.activation(
                                ao[:, ts(h, 64)],
                                avp[qt // 2][:, 4 * (qt % 2) + h, :],
                                COPY, scale=rec[:, qtg, h:h + 1])
                    for fc in range(FT):
                        nc.sync.dma_start_transpose(
                            aoT[:, fc, qtg, :], ao[:, ts(fc, 128)])

            # ---- emission ----
            warm = next_sc("warm")
            for i in range(14):
                nc.tensor.matmul(warm[:], wh_sb[:, 0, :, 0:128],
                                 xh_sb[:, 0, :, 0:512], start=True,
                                 stop=True, perf_mode=DR)
            pending = []
            proj_row(2)   # K heads 0,1
            proj_row(0)   # Q heads 0,1
            avp = [ps2.tile([128, 8, 64], F32, tag=f"av{j}",
                            name=f"avp0_{j}") for j in range(2)]
            den = ps2.tile([128, 16], F32, tag="den", name="den0")
            touches = {"av0": 0, "av1": 0, "den": 0}
            tilesA = [(kt, h) for kt in range(TT) for h in (0, 1)]

            def extraA(i):
                if i % 2 == 0 and i // 2 < TT:
                    emit_v(i // 2)
                if i == 8:
                    proj_row(3)   # K heads 2,3
                if i == 20:
                    proj_row(1)   # Q heads 2,3
            run_tiles(0, tilesA, avp, den, extra=extraA)
            tilesB = [(kt, h) for kt in range(TT) for h in (2, 3)]
            run_tiles(0, tilesB, avp, den)
            qb_tail(0, avp, den)
            pending += [(0, qt) for qt in range(4)]
            for qb in range(1, QB):
                avp = [ps2.tile([128, 8, 64], F32, tag=f"av{j}",
                                name=f"avp{qb}_{j}") for j in range(2)]
                den = ps2.tile([128, 16], F32, tag="den", name=f"den{qb}")
                touches = {"av0": 0, "av1": 0, "den": 0}
                tiles = [(kt, h) for kt in range(TT) for h in range(HLOC)]

                def extraQ(i):
                    if pending and i in (8, 16, 24, 32):
                        pq_, pt_ = pending.pop(0)
                        emit_po(pq_, pt_, i)
                run_tiles(qb, tiles, avp, den, extra=extraQ)
                qb_tail(qb, avp, den)
                pending += [(qb, qt) for qt in range(4)]
            for j, (pq_, pt_) in enumerate(pending):
                emit_po(pq_, pt_, j * 2)


_CACHED = {}


def _get_module():
    if "nc" not in _CACHED:
        nc = bacc.Bacc("TRN2")
        build_mha(nc)
        nc.finalize()
        _CACHED["nc"] = nc
    return _CACHED["nc"]


def _f8(x):
    import ml_dtypes
    return np.asarray(x, np.float32).astype(ml_dtypes.float8_e4m3)


def _bf(x):
    import ml_dtypes
    return np.asarray(x, np.float32).astype(ml_dtypes.bfloat16)


def make_in_maps(query, w_in, b_in, w_o):
    E, FLoc = D_MODEL, FL
    ECP = E // 256
    scale = 0.125
    wq, wk, wv = w_in[:E], w_in[E:2 * E], w_in[2 * E:]
    bq = np.asarray(b_in[:E], np.float32) * scale
    bv = np.asarray(b_in[2 * E:], np.float32)
    in_maps = []
    ones = np.ones((1, 512), np.float32)
    for core in range(N_CORES):
        b, g = divmod(core, GROUPS)
        sl = slice(g * FLoc, (g + 1) * FLoc)
        wcat = np.concatenate([
            np.asarray(wq[sl], np.float32).T * (scale * SWQ),
            np.asarray(wk[sl], np.float32).T * SWK,
            np.asarray(wv[sl], np.float32).T * SWV], axis=1)  # (E, 768)
        whh = _f8(wcat)
        wll = _f8(wcat - whh.astype(np.float32))
        xt = np.asarray(query[b], np.float32).T * SX          # (E, S)
        xhh = _f8(xt)
        xll = _f8(xt - xhh.astype(np.float32))

        def fold(a):  # (E, F) -> (128, ECP, 2, F)
            return np.ascontiguousarray(
                a.reshape(ECP, 2, 128, -1).transpose(2, 0, 1, 3))
        qbias = (bq[sl] * SX * SWQ)[None, :]
        vb_a = (bv[sl] * SX * SWV)[None, :]
        woT_a = np.ascontiguousarray(
            np.asarray(w_o, np.float32).T[sl].reshape(FL // 128, 128, E)
            .transpose(1, 0, 2))
        in_maps.append({
            "xh": fold(xhh), "xl": fold(xll),
            "wh": fold(whh), "wl": fold(wll),
            "qb8": _f8(qbias), "ones8": _f8(ones),
            "vb8": _f8(vb_a), "woT": _bf(woT_a),
        })
    return in_maps


def kernel(query, key, value, w_in, b_in, w_o, b_o, _trace=False):
    from concourse.bass_utils import run_bass_kernel_spmd
    nc = _get_module()
    in_maps = make_in_maps(np.asarray(query), np.asarray(w_in),
                           np.asarray(b_in), np.asarray(w_o))
    res = run_bass_kernel_spmd(nc, in_maps, core_ids=list(range(N_CORES)),
                               trace=_trace)
    out = np.empty((N_BATCH, SEQ, D_MODEL), np.float32)
    for b in range(N_BATCH):
        acc = res.results[b * GROUPS]["out"].astype(np.float32)
        for g in range(1, GROUPS):
            acc = acc + res.results[b * GROUPS + g]["out"]
        out[b] = acc + np.asarray(b_o, np.float32)[None, :]
    if _trace:
        kernel.last_exec_time_ns = res.exec_time_ns
    return out
